# revision 7
# baseline (speedup 1.0000x reference)
"""Trainium2 Bass kernel for nn_ContextInteractionModel.

Data-parallel: batch dim (512) sharded across 8 NeuronCores (64 each);
embed_table / att_mat replicated. Per batch b on-device:
    A = E[idx1_b] (128,512), B = E[idx2_b]
    ssA = sum(A^2), ssB = sum(B^2)            (Frobenius norms, fused accum)
    C^T = att^T @ A^T                          (PE transposes + fp32r matmuls)
    S   = tanh((C @ B^T) * rsqrt(ssA*ssB))     (fp32r matmuls + ACT tanh)
    rows_u = exp(mean_m S), cols_u = exp(mean_l S)   (unnormalized softmax)
    newA_u = A^T rows_u, newB_u = B^T cols_u   (fp32 matvecs, d on partitions)
    stage prod = newA_u*newB_u, Ra=sum rows_u, Rb=sum cols_u, ssA, ssB
Host: logits_b = (prod @ w) / (sqrt(ssA*ssB)*Ra*Rb) + b_pred.
(The tiny 512-elem dot with w is O(B*D) on host; all O(B*L*D^2) work on device.)
"""
import sys
import numpy as np

sys.path.insert(0, '/opt/trn_rl_repo')

from contextlib import ExitStack

import concourse.bass as bass
import concourse.tile as tile
from concourse import mybir, bacc
from concourse.bass_utils import run_bass_kernel_spmd
from concourse.masks import make_identity

B, L, D, V = 512, 128, 512, 100000
NCORES = 8
BPC = B // NCORES          # 64 batches per core
CHUNK = 8                  # gather/norm chunk (batches)
NCHUNK = BPC // CHUNK
PAIR = 2                   # matmul group (batches) — bounded by PSUM banks
F32 = mybir.dt.float32
F32R = mybir.dt.float32r
I32 = mybir.dt.int32

RSQRT_MAGIC = 0x5F3759DF


def _build_nc():
    nc = bacc.Bacc("TRN2", target_bir_lowering=False, debug=False,
                   num_devices=NCORES)
    E = nc.dram_tensor("embed", [V, D], F32, kind="ExternalInput").ap()
    att = nc.dram_tensor("att", [D, D], F32, kind="ExternalInput").ap()
    idx1 = nc.dram_tensor("idx1", [BPC, L], I32, kind="ExternalInput").ap()
    idx2 = nc.dram_tensor("idx2", [BPC, L], I32, kind="ExternalInput").ap()
    s_out = nc.dram_tensor("s_out", [BPC, L, L], F32, kind="ExternalOutput").ap()
    # prod staged partition-major: fin_prod[p, b, dt] = (newA_u*newB_u)[b, dt*128+p]
    fin_prod = nc.dram_tensor("fin_prod", [128, BPC, 4], F32,
                              kind="ExternalOutput").ap()
    # fin_scal[0, 4b:4b+4] = [Ra, Rb, ssA, ssB]
    fin_scal = nc.dram_tensor("fin_scal", [1, 4 * BPC], F32,
                              kind="ExternalOutput").ap()

    with tile.TileContext(nc) as tc, ExitStack() as ctx:
        _body(nc, tc, ctx, E, att, idx1, idx2, s_out, fin_prod, fin_scal)
    nc.compile()
    return nc


def _body(nc, tc, ctx, E, att, idx1, idx2, s_out, fin_prod, fin_scal):
    P = 128
    consts = ctx.enter_context(tc.tile_pool(name="consts", bufs=1))
    gath = ctx.enter_context(tc.tile_pool(name="gath", bufs=2))
    work = ctx.enter_context(tc.tile_pool(name="work", bufs=2))
    small = ctx.enter_context(tc.tile_pool(name="small", bufs=2))

    ps_ct = ctx.enter_context(tc.tile_pool(name="ps_ct", bufs=1, space="PSUM"))
    ps_t = ctx.enter_context(tc.tile_pool(name="ps_t", bufs=2, space="PSUM"))
    ps_s = ctx.enter_context(tc.tile_pool(name="ps_s", bufs=1, space="PSUM"))
    ps_nab = ctx.enter_context(tc.tile_pool(name="ps_nab", bufs=1, space="PSUM"))
    ps_etc = ctx.enter_context(tc.tile_pool(name="ps_etc", bufs=2, space="PSUM"))

    # ---------------- constants ----------------
    ident = consts.tile([P, P], F32)
    make_identity(nc, ident[:])
    ones = consts.tile([P, P], F32)
    nc.vector.memset(ones[:], 1.0)
    # att tiles: att[d,e] natural layout, 4 partition-tiles of [128, 512],
    # rounded to fp32r for the M1 matmuls
    att_f = consts.tile([P, 4 * D], F32)
    att_r = consts.tile([P, 4 * D], F32R)
    for dt_ in range(4):
        nc.sync.dma_start(out=att_f[:, dt_ * D:(dt_ + 1) * D],
                          in_=att[dt_ * P:(dt_ + 1) * P, :])
    nc.vector.tensor_copy(out=att_r[:], in_=att_f[:])

    # staging accumulators (written all run, stored once at the end)
    prod_all = consts.tile([P, 4 * BPC], F32)
    scal_all = consts.tile([1, 4 * BPC], F32)

    # ---------------- index preload & transpose ----------------
    # [64,128] natural -> f32 -> PE transpose -> [128,64] int32
    idxT = []
    for k, idx in enumerate((idx1, idx2)):
        nat = small.tile([BPC, L], I32, tag="idxnat")
        nc.sync.dma_start(out=nat[:], in_=idx[:, :])
        natf = small.tile([BPC, L], F32, tag="idxnatf")
        nc.vector.tensor_copy(out=natf[:], in_=nat[:])
        tps = ps_etc.tile([P, BPC], F32, space="PSUM", tag="etc")
        nc.tensor.transpose(out=tps[:], in_=natf[:], identity=ident[0:BPC, 0:BPC])
        it = consts.tile([L, BPC], I32, name=f"idxT{k}")
        nc.vector.tensor_copy(out=it[:], in_=tps[:])
        idxT.append(it)

    inv_L = 1.0 / L

    for c in range(NCHUNK):
        b0 = c * CHUNK
        # ---------------- gather chunk ----------------
        A_ch = gath.tile([P, CHUNK * D], F32, tag="ach")
        B_ch = gath.tile([P, CHUNK * D], F32, tag="bch")
        for j in range(CHUNK):
            nc.gpsimd.indirect_dma_start(
                out=A_ch[:, j * D:(j + 1) * D], out_offset=None, in_=E[:, :],
                in_offset=bass.IndirectOffsetOnAxis(
                    ap=idxT[0][:, b0 + j:b0 + j + 1], axis=0))
            nc.gpsimd.indirect_dma_start(
                out=B_ch[:, j * D:(j + 1) * D], out_offset=None, in_=E[:, :],
                in_offset=bass.IndirectOffsetOnAxis(
                    ap=idxT[1][:, b0 + j:b0 + j + 1], axis=0))

        # ---------------- norms: ssq[:, 2j]=per-row sum A_j^2, 2j+1 for B ---
        ssq = small.tile([P, 2 * CHUNK], F32, tag="ssq")
        junk_a = work.tile([P, D], F32, tag="junka", bufs=1)
        junk_d = work.tile([P, D], F32, tag="junkd", bufs=1)
        for j in range(CHUNK):
            nc.scalar.activation(
                out=junk_a[:], in_=A_ch[:, j * D:(j + 1) * D],
                func=mybir.ActivationFunctionType.Square,
                accum_out=ssq[:, 2 * j:2 * j + 1])
            nc.scalar.activation(
                out=junk_d[:], in_=B_ch[:, j * D:(j + 1) * D],
                func=mybir.ActivationFunctionType.Square,
                accum_out=ssq[:, 2 * j + 1:2 * j + 2])
        # partition-sum + broadcast to all 128 partitions: ones^T @ ssq
        ns_ps = ps_etc.tile([P, 2 * CHUNK], F32, space="PSUM", tag="etc")
        nc.tensor.matmul(out=ns_ps[:], lhsT=ones[:], rhs=ssq[:],
                         start=True, stop=True)
        ns_sb = small.tile([P, 2 * CHUNK], F32, tag="nssb")
        nc.vector.tensor_copy(out=ns_sb[:], in_=ns_ps[:])

        # rn = rsqrt(ssA*ssB): bit-trick seed + 3 Newton iters (DVE only)
        pp = small.tile([P, CHUNK], F32, tag="pp")
        nc.vector.tensor_tensor(
            out=pp[:], in0=ns_sb[:, 0:2 * CHUNK:2], in1=ns_sb[:, 1:2 * CHUNK:2],
            op=mybir.AluOpType.mult)
        rn = small.tile([P, CHUNK], F32, tag="rn")
        t1 = small.tile([P, CHUNK], F32, tag="nt1")
        t2 = small.tile([P, CHUNK], F32, tag="nt2")
        nc.vector.tensor_scalar(
            out=t1[:].bitcast(I32), in0=pp[:].bitcast(I32),
            scalar1=1, scalar2=None, op0=mybir.AluOpType.logical_shift_right)
        nc.vector.tensor_scalar(
            out=rn[:].bitcast(I32), in0=t1[:].bitcast(I32),
            scalar1=-1, scalar2=RSQRT_MAGIC,
            op0=mybir.AluOpType.mult, op1=mybir.AluOpType.add)
        for _ in range(3):
            nc.vector.tensor_tensor(out=t1[:], in0=rn[:], in1=rn[:],
                                    op=mybir.AluOpType.mult)
            nc.vector.tensor_tensor(out=t2[:], in0=t1[:], in1=pp[:],
                                    op=mybir.AluOpType.mult)
            nc.vector.tensor_scalar(out=t1[:], in0=t2[:],
                                    scalar1=-0.5, scalar2=1.5,
                                    op0=mybir.AluOpType.mult,
                                    op1=mybir.AluOpType.add)
            nc.vector.tensor_tensor(out=rn[:], in0=rn[:], in1=t1[:],
                                    op=mybir.AluOpType.mult)

        S_sb = work.tile([P, CHUNK * L], F32, tag="ssb")
        rowsum = small.tile([P, CHUNK], F32, tag="rowsum")
        ru_cu = small.tile([P, 2 * CHUNK], F32, tag="rucu")

        for p in range(CHUNK // PAIR):
            jb0 = p * PAIR          # chunk-local batch of this pair
            # ------------- transposes: A^T, B^T for the pair (-> fp32r) ----
            At = work.tile([P, PAIR * D], F32R, tag="atp")
            Bt = work.tile([P, PAIR * D], F32R, tag="btp")
            for jj in range(PAIR):
                src_a = A_ch[:, (jb0 + jj) * D:(jb0 + jj + 1) * D]
                src_b = B_ch[:, (jb0 + jj) * D:(jb0 + jj + 1) * D]
                tp_a = ps_t.tile([P, D], F32, space="PSUM", tag="tps")
                for dt_ in range(4):
                    nc.tensor.transpose(out=tp_a[:, dt_ * P:(dt_ + 1) * P],
                                        in_=src_a[:, dt_ * P:(dt_ + 1) * P],
                                        identity=ident[:])
                nc.vector.tensor_copy(out=At[:, jj * D:(jj + 1) * D], in_=tp_a[:])
                tp_b = ps_t.tile([P, D], F32, space="PSUM", tag="tps")
                for dt_ in range(4):
                    nc.tensor.transpose(out=tp_b[:, dt_ * P:(dt_ + 1) * P],
                                        in_=src_b[:, dt_ * P:(dt_ + 1) * P],
                                        identity=ident[:])
                nc.scalar.copy(out=Bt[:, jj * D:(jj + 1) * D], in_=tp_b[:])

            # ------------- M1: C^T[e-tile] = sum_dt att^T A^T -------------
            ct_ps = ps_ct.tile([P, PAIR * D], F32, space="PSUM", tag="ct")
            At_v = At[:].rearrange("p (b t l) -> p b t l", b=PAIR, t=4)
            Bt_v = Bt[:].rearrange("p (b t l) -> p b t l", b=PAIR, t=4)
            for et in range(4):
                for dt_ in range(4):
                    nc.tensor.matmul(
                        out=ct_ps[:, et * PAIR * P:(et + 1) * PAIR * P],
                        lhsT=att_r[:, dt_ * D + et * P:dt_ * D + (et + 1) * P],
                        rhs=At_v[:, :, dt_, :],
                        start=(dt_ == 0), stop=(dt_ == 3))
            ct_sb = work.tile([P, PAIR * D], F32R, tag="cts")
            nc.vector.tensor_copy(out=ct_sb[:, :PAIR * D // 2],
                                  in_=ct_ps[:, :PAIR * D // 2])
            nc.scalar.copy(out=ct_sb[:, PAIR * D // 2:],
                           in_=ct_ps[:, PAIR * D // 2:])

            # ------------- M2: S = C @ B^T (pair-batched streams) ---------
            s_ps = ps_s.tile([P, PAIR * 256], F32, space="PSUM", tag="sps")
            for jj in range(PAIR):
                for et in range(4):
                    nc.tensor.matmul(
                        out=s_ps[:, jj * 256:(jj + 1) * 256],
                        lhsT=ct_sb[:, et * PAIR * P + jj * P:
                                   et * PAIR * P + (jj + 1) * P],
                        rhs=Bt_v[:, :, et, :],
                        start=(et == 0), stop=(et == 3))

            # ------------- per-batch epilogue -------------
            for jj in range(PAIR):
                j = jb0 + jj        # chunk-local batch index
                bg = b0 + j         # core-local batch index
                s_use = s_ps[:, jj * 256 + jj * L:jj * 256 + (jj + 1) * L]
                nc.scalar.activation(
                    out=S_sb[:, j * L:(j + 1) * L], in_=s_use,
                    func=mybir.ActivationFunctionType.Tanh,
                    scale=rn[:, j:j + 1],
                    accum_out=rowsum[:, j:j + 1])
                # rows_u = exp(rowsum/L)
                nc.scalar.activation(
                    out=ru_cu[:, 2 * j:2 * j + 1], in_=rowsum[:, j:j + 1],
                    func=mybir.ActivationFunctionType.Exp, scale=inv_L)
                # colsum[m] = S^T @ ones
                cs_ps = ps_etc.tile([P, 1], F32, space="PSUM", tag="etc")
                nc.tensor.matmul(out=cs_ps[:],
                                 lhsT=S_sb[:, j * L:(j + 1) * L],
                                 rhs=ones[:, 0:1], start=True, stop=True)
                nc.scalar.activation(
                    out=ru_cu[:, 2 * j + 1:2 * j + 2], in_=cs_ps[:],
                    func=mybir.ActivationFunctionType.Exp, scale=inv_L)
                # newA_u/newB_u: lhsT = gathered slices (fp32), rhs = [ru|cu]
                # out[128, 2] per d-tile: col 0 = A^T rows_u, col 1 = B^T cols_u
                nab_ps = ps_nab.tile([P, 16], F32, space="PSUM", tag="nab")
                for dt_ in range(4):
                    nc.tensor.matmul(
                        out=nab_ps[:, 2 * dt_:2 * dt_ + 2],
                        lhsT=A_ch[:, j * D + dt_ * P:j * D + (dt_ + 1) * P],
                        rhs=ru_cu[:, 2 * j:2 * j + 2],
                        start=True, stop=True)
                    nc.tensor.matmul(
                        out=nab_ps[:, 8 + 2 * dt_:8 + 2 * dt_ + 2],
                        lhsT=B_ch[:, j * D + dt_ * P:j * D + (dt_ + 1) * P],
                        rhs=ru_cu[:, 2 * j:2 * j + 2],
                        start=True, stop=True)
                # prod[p, dt] = newA_u[dt*128+p] * newB_u[dt*128+p]
                nab_sb = small.tile([P, 16], F32, tag="nabsb")
                nc.vector.tensor_copy(out=nab_sb[:], in_=nab_ps[:])
                nc.vector.tensor_tensor(
                    out=prod_all[:, 4 * bg:4 * bg + 4],
                    in0=nab_sb[:, 0:8:2], in1=nab_sb[:, 9:16:2],
                    op=mybir.AluOpType.mult)
                # Ra, Rb = partition sums of rows_u / cols_u -> [1, 2]
                rr_ps = ps_etc.tile([1, 2], F32, space="PSUM", tag="etc")
                nc.tensor.matmul(out=rr_ps[:], lhsT=ones[:, 0:1],
                                 rhs=ru_cu[:, 2 * j:2 * j + 2],
                                 start=True, stop=True)
                nc.scalar.copy(out=scal_all[0:1, 4 * bg:4 * bg + 2],
                               in_=rr_ps[0:1, :])
                nc.vector.tensor_copy(out=scal_all[0:1, 4 * bg + 2:4 * bg + 4],
                                      in_=ns_sb[0:1, 2 * j:2 * j + 2])

        # ---------------- store S for the chunk ----------------
        s_dst = s_out[b0:b0 + CHUNK, :, :].transpose([1, 0, 2])
        nc.sync.dma_start(out=s_dst, in_=S_sb[:].rearrange(
            "p (b m) -> p b m", b=CHUNK))

    # ---------------- final staging stores ----------------
    nc.sync.dma_start(out=fin_prod[:, :, :],
                      in_=prod_all[:].rearrange("p (b t) -> p b t", b=BPC))
    nc.sync.dma_start(out=fin_scal[:, :], in_=scal_all[:])


_NC_CACHE = None


def _get_nc():
    global _NC_CACHE
    if _NC_CACHE is None:
        _NC_CACHE = _build_nc()
    return _NC_CACHE


def _run(inputs, trace=False):
    t1c = np.ascontiguousarray(np.asarray(inputs["t1_contexts"], dtype=np.int32))
    t2c = np.ascontiguousarray(np.asarray(inputs["t2_contexts"], dtype=np.int32))
    E = np.ascontiguousarray(np.asarray(inputs["embed_table"], dtype=np.float32))
    att = np.ascontiguousarray(np.asarray(inputs["att_mat"], dtype=np.float32))
    w = np.asarray(inputs["w_pred"], dtype=np.float32).reshape(-1)
    bp = np.asarray(inputs["b_pred"], dtype=np.float32).reshape(-1)

    nc = _get_nc()
    in_maps = []
    for i in range(NCORES):
        sl = slice(i * BPC, (i + 1) * BPC)
        in_maps.append({
            "embed": E, "att": att,
            "idx1": np.ascontiguousarray(t1c[sl]),
            "idx2": np.ascontiguousarray(t2c[sl]),
        })
    res = run_bass_kernel_spmd(nc, in_maps, core_ids=list(range(NCORES)),
                               trace=trace)
    S = np.concatenate([res.results[i]["s_out"] for i in range(NCORES)], axis=0)
    # fin_prod[p, b, t] -> prod[b, t*128+p]
    prod = np.concatenate(
        [res.results[i]["fin_prod"].transpose(1, 2, 0).reshape(BPC, D)
         for i in range(NCORES)], axis=0)
    scal = np.concatenate(
        [res.results[i]["fin_scal"].reshape(BPC, 4) for i in range(NCORES)],
        axis=0)
    Ra, Rb, ssA, ssB = scal[:, 0], scal[:, 1], scal[:, 2], scal[:, 3]
    logits = (prod @ w) / (np.sqrt(ssA.astype(np.float64) * ssB.astype(np.float64))
                           * Ra * Rb).astype(np.float32) + bp[0]
    return (logits.astype(np.float32), S), res


def kernel(**inputs):
    out, _ = _run(inputs, trace=False)
    return out


def kernel_traced(**inputs):
    out, res = _run(inputs, trace=True)
    return out, res
